# revision 1
# baseline (speedup 1.0000x reference)
"""Soft-MoE forward on 8 TRN2 NeuronCores.

Strategy: pure data-parallel over the batch axis (B=16 -> 2 batches/core).
Each core runs router + dispatch + all-expert GLU-MLP + combine for its two
batches; expert weights are replicated (streamed from HBM in bf16). No
cross-core communication is needed because the expert MLP is row-independent:
each batch contributes CAP=128 rows per expert, so a core can apply all 16
experts to its own batches' rows.

All matmuls run in bf16 (PE full rate); softmax statistics in fp32.
exp() skips max-subtraction: logits are ~N(0, 0.64^2), so exp is well within
fp32/bf16 range.
"""

import numpy as np
import ml_dtypes

import concourse.bass as bass
import concourse.tile as tile
from concourse import mybir

B, T, C, E, H = 16, 2048, 1024, 16, 2048
CAP = T // E  # 128
S = E * CAP  # 2048 slots
P = 128  # partitions
NCORES = 8
BL = B // NCORES  # 2 batches per core

FP32 = mybir.dt.float32
BF16 = mybir.dt.bfloat16
AX = mybir.AluOpType


def _split_multi_waits(nc):
    """This walrus build accepts only ONE sync wait per instruction; Tile's
    wait-assignment can emit several. Move extra waits onto single-wait nops
    inserted just before the instruction on the same engine (same-engine
    program order makes this semantically identical)."""
    import bass_rust

    nid = 0
    for f in nc.m.functions:
        for bb in f.blocks:
            out = []
            changed = False
            for inst in bb.instructions:
                si = inst.sync_info
                waits = list(si.on_wait) if si and si.on_wait else []
                if len(waits) > 1:
                    changed = True
                    for w in waits[:-1]:
                        nop = mybir.InstNoOp(name=f"TW-{nid}", ins=[], outs=[])
                        nid += 1
                        nop.engine = inst.engine
                        nop.sync_info = bass_rust.SyncInfo(
                            on_wait=[w], on_update=[]
                        )
                        out.append(nop)
                    si.on_wait = waits[-1:]
                out.append(inst)
            if changed:
                bb.instructions = out


def build_nc(loops=BL):
    assert loops % BL == 0
    nc = bass.Bass(trn_type="TRN2")

    xb = nc.dram_tensor("xb", [BL, T, C], BF16, kind="ExternalInput")
    xbt = nc.dram_tensor("xbt", [BL, C, T], BF16, kind="ExternalInput")
    wrt = nc.dram_tensor("wrt", [C, S], BF16, kind="ExternalInput")
    wg = nc.dram_tensor("wg", [E, C, H], BF16, kind="ExternalInput")
    wf = nc.dram_tensor("wf", [E, C, H], BF16, kind="ExternalInput")
    wp = nc.dram_tensor("wp", [E, H, C], BF16, kind="ExternalInput")
    y = nc.dram_tensor("y", [BL, T, C], FP32, kind="ExternalOutput")

    Tt = T // P  # 16 T tiles
    Ct = C // P  # 8 C tiles
    St = S // P  # 16 slot tiles (= experts)
    Ht = H // P  # 16 H tiles

    with tile.TileContext(nc) as tc:
        with (
            tc.tile_pool(name="dram", bufs=2, space="DRAM") as dpool,
            tc.tile_pool(name="const", bufs=1) as cpool,
            tc.tile_pool(name="io3", bufs=3) as p3,
            tc.tile_pool(name="io2", bufs=2) as p2,
            tc.tile_pool(name="stat", bufs=4) as sp,
            tc.tile_pool(name="psum", bufs=8, space="PSUM") as pp,
        ):
            for rep in range(loops // BL):
                # router weights resident: [128, Ct, S]; tag shared with eo
                # (wrt is dead by the time eo tiles are written)
                wrt_sb = cpool.tile([P, Ct, S], BF16, tag="big", bufs=2, name="wrt_sb")
                nc.sync.dma_start(
                    wrt_sb[:], wrt.rearrange("(c p) s -> p c s", p=P)
                )
                p_drams, pt_drams, eit_drams, rzcs, rzds = {}, {}, {}, {}, {}
                for b in range(BL):
                    p_dram = dpool.tile([T, S], BF16, tag="p_dram", name=f"pd{b}")
                    pt_dram = dpool.tile([S, T], BF16, tag="pt_dram", name=f"ptd{b}")
                    eit_dram = dpool.tile([C, S], BF16, tag="eit_dram", name=f"eitd{b}")
                    rzc = p2.tile([P, Tt], FP32, tag="rzc", name=f"rzc{b}")
                    zdall = p2.tile([P, Tt, St], FP32, tag="zdall", name=f"zdall{b}")
                    rzd = p2.tile([P, St], FP32, tag="rzd", name=f"rzd{b}")
                    p_drams[b], pt_drams[b], eit_drams[b] = p_dram, pt_dram, eit_dram
                    rzcs[b], rzds[b] = rzc, rzd

                    # ---- Phase R: router logits g = x @ wr^T, P = exp(g) ----
                    for t in range(Tt):
                        xbt_t = p3.tile([P, Ct, P], BF16, tag="xbt_t")
                        nc.sync.dma_start(
                            xbt_t[:],
                            xbt[b].rearrange("(c p) t -> p c t", p=P)[
                                :, :, t * P : (t + 1) * P
                            ],
                        )
                        gps = [pp.tile([P, 512], FP32, tag="ps", name=f"gps{n}") for n in range(4)]
                        for c in range(Ct):
                            for n in range(4):
                                nc.tensor.matmul(
                                    gps[n][:],
                                    xbt_t[:, c, :],
                                    wrt_sb[:, c, n * 512 : (n + 1) * 512],
                                    start=(c == 0),
                                    stop=(c == Ct - 1),
                                )
                        pch = p3.tile([P, S], BF16, tag="pch")
                        zc4 = sp.tile([P, 4], FP32, tag="zc4")
                        for n in range(4):
                            nc.scalar.activation(
                                pch[:, n * 512 : (n + 1) * 512],
                                gps[n][:],
                                mybir.ActivationFunctionType.Exp,
                                accum_out=zc4[:, n : n + 1],
                            )
                        zc1 = sp.tile([P, 1], FP32, tag="zc1")
                        nc.vector.tensor_reduce(zc1[:], zc4[:], mybir.AxisListType.X, AX.add)
                        nc.vector.reciprocal(rzc[:, t : t + 1], zc1[:])
                        nc.gpsimd.dma_start(
                            p_dram[t * P : (t + 1) * P, :], pch[:]
                        )

                    # ---- Phase T: transpose P -> PT (DMA xbar), Zd partials ----
                    for t in range(Tt):
                        ptt = p3.tile([P, St, P], BF16, tag="ptt")
                        nc.scalar.dma_start_transpose(
                            ptt[:], p_dram[t * P : (t + 1) * P, :]
                        )
                        nc.vector.tensor_reduce(
                            zdall[:, t, :], ptt[:], mybir.AxisListType.X, AX.add
                        )
                        nc.gpsimd.dma_start(
                            pt_dram.rearrange("(s p) t -> p s t", p=P)[
                                :, :, t * P : (t + 1) * P
                            ],
                            ptt[:],
                        )
                    zd = sp.tile([P, St], FP32, tag="zd")
                    nc.vector.tensor_reduce(
                        zd[:], zdall.rearrange("p t s -> p s t"), mybir.AxisListType.X, AX.add
                    )
                    nc.vector.reciprocal(rzd[:], zd[:])

                    # ---- Phase D: dispatch eiT = x^T @ P (unnormalized) ----
                    for nh in range(2):
                        for mg in range(2):
                            dps = [pp.tile([P, 512], FP32, tag="ps", name=f"dps{i}") for i in range(8)]
                            for k in range(Tt):
                                xk = p3.tile([P, 512], BF16, tag="xk")
                                nc.sync.dma_start(
                                    xk[:],
                                    xb[b].rearrange("(k p) c -> p k c", p=P)[
                                        :, k, mg * 512 : (mg + 1) * 512
                                    ],
                                )
                                pk = p3.tile([P, 1024], BF16, tag="pk")
                                nc.sync.dma_start(
                                    pk[:],
                                    p_dram[
                                        k * P : (k + 1) * P,
                                        nh * 1024 : (nh + 1) * 1024,
                                    ],
                                )
                                for m4 in range(4):
                                    for n2 in range(2):
                                        nc.tensor.matmul(
                                            dps[m4 * 2 + n2][:],
                                            xk[:, m4 * P : (m4 + 1) * P],
                                            pk[:, n2 * 512 : (n2 + 1) * 512],
                                            start=(k == 0),
                                            stop=(k == Tt - 1),
                                        )
                            for i in range(8):
                                m = mg * 4 + i // 2
                                n = nh * 2 + i % 2
                                et = p3.tile([P, 512], BF16, tag="et")
                                nc.scalar.copy(et[:], dps[i][:])
                                nc.gpsimd.dma_start(
                                    eit_dram[
                                        m * P : (m + 1) * P, n * 512 : (n + 1) * 512
                                    ],
                                    et[:],
                                )

                # ---- Phase M: per-expert GLU MLP over BOTH batches ----
                eos = {
                    b: cpool.tile([P, St, C], BF16, tag="big", bufs=2, name=f"eo{b}")
                    for b in range(BL)
                }
                for e in range(E):
                    eites = {}
                    for b in range(BL):
                        eit_e = p2.tile([P, Ct, P], BF16, tag="eit_e", name=f"eite{b}")
                        nc.sync.dma_start(
                            eit_e[:],
                            eit_drams[b].rearrange("(c p) s -> p c s", p=P)[
                                :, :, e * P : (e + 1) * P
                            ],
                        )
                        eites[b] = eit_e
                    hs = {
                        b: p2.tile([P, H], BF16, tag="h_sb", name=f"h{b}")
                        for b in range(BL)
                    }
                    for hc in range(4):
                        wgc = p2.tile([P, Ct, 512], BF16, tag="wgc")
                        nc.sync.dma_start(
                            wgc[:],
                            wg[e].rearrange("(c p) h -> p c h", p=P)[
                                :, :, hc * 512 : (hc + 1) * 512
                            ],
                        )
                        wfcc = p2.tile([P, Ct, 512], BF16, tag="wfcc")
                        nc.sync.dma_start(
                            wfcc[:],
                            wf[e].rearrange("(c p) h -> p c h", p=P)[
                                :, :, hc * 512 : (hc + 1) * 512
                            ],
                        )
                        for b in range(BL):
                            gg = pp.tile([P, 512], FP32, tag="ps", name=f"gg{b}")
                            hh = pp.tile([P, 512], FP32, tag="ps", name=f"hh{b}")
                            for c in range(Ct):
                                nc.tensor.matmul(
                                    gg[:], eites[b][:, c, :], wgc[:, c, :],
                                    start=(c == 0), stop=(c == Ct - 1),
                                )
                            for c in range(Ct):
                                nc.tensor.matmul(
                                    hh[:], eites[b][:, c, :], wfcc[:, c, :],
                                    start=(c == 0), stop=(c == Ct - 1),
                                )
                            sg = p3.tile([P, 512], BF16, tag="sg", name=f"sg{b}")
                            nc.scalar.activation(
                                sg[:], gg[:],
                                mybir.ActivationFunctionType.Silu,
                                scale=rzds[b][:, e : e + 1],
                            )
                            nc.vector.scalar_tensor_tensor(
                                hs[b][:, hc * 512 : (hc + 1) * 512],
                                hh[:], rzds[b][:, e : e + 1], sg[:],
                                AX.mult, AX.mult,
                            )
                    hts = {}
                    for b in range(BL):
                        ht = p2.tile([P, Ht, P], BF16, tag="ht", name=f"ht{b}")
                        nc.scalar.dma_start_transpose(ht[:], hs[b][:])
                        hts[b] = ht
                    for cc in range(4):
                        wpc = p2.tile([P, Ht, 256], BF16, tag="wpc")
                        nc.sync.dma_start(
                            wpc[:],
                            wp[e].rearrange("(k p) c -> p k c", p=P)[
                                :, :, cc * 256 : (cc + 1) * 256
                            ],
                        )
                        for b in range(BL):
                            eop = pp.tile([P, 256], FP32, tag="ps", name=f"eop{b}")
                            for k in range(Ht):
                                nc.tensor.matmul(
                                    eop[:], hts[b][:, k, :], wpc[:, k, :],
                                    start=(k == 0), stop=(k == Ht - 1),
                                )
                            nc.scalar.copy(
                                eos[b][:, e, cc * 256 : (cc + 1) * 256], eop[:]
                            )

                # ---- Phase C: combine y = (P @ eo) * rzc ----
                for b in range(BL):
                    for t in range(Tt):
                        ptr = p2.tile([P, St, P], BF16, tag="ptr")
                        nc.sync.dma_start(
                            ptr[:],
                            pt_drams[b].rearrange("(s p) t -> p s t", p=P)[
                                :, :, t * P : (t + 1) * P
                            ],
                        )
                        for cc in range(2):
                            yps = pp.tile([P, 512], FP32, tag="ps", name="yps")
                            for e in range(St):
                                nc.tensor.matmul(
                                    yps[:],
                                    ptr[:, e, :],
                                    eos[b][:, e, cc * 512 : (cc + 1) * 512],
                                    start=(e == 0),
                                    stop=(e == St - 1),
                                )
                            ysb = p3.tile([P, 512], FP32, tag="ysb")
                            nc.vector.tensor_scalar_mul(
                                ysb[:], yps[:], rzcs[b][:, t : t + 1]
                            )
                            nc.gpsimd.dma_start(
                                y[b, t * P : (t + 1) * P, cc * 512 : (cc + 1) * 512],
                                ysb[:],
                            )
    _split_multi_waits(nc)
    return nc


def kernel(x, w_router_gate, w_fc, w_gate, w_proj):
    bf16 = ml_dtypes.bfloat16
    wrt_np = np.ascontiguousarray(
        w_router_gate.reshape(S, C).T
    ).astype(bf16)
    wg_np = w_gate.astype(bf16)
    wf_np = w_fc.astype(bf16)
    wp_np = w_proj.astype(bf16)

    in_maps = []
    for c in range(NCORES):
        xc = x[c * BL : (c + 1) * BL]  # [BL, T, C] fp32
        xb_np = xc.astype(bf16)
        xbt_np = np.ascontiguousarray(xb_np.transpose(0, 2, 1))
        in_maps.append(
            {
                "xb": xb_np,
                "xbt": xbt_np,
                "wrt": wrt_np,
                "wg": wg_np,
                "wf": wf_np,
                "wp": wp_np,
            }
        )

    from concourse.bass_utils import run_bass_kernel_spmd

    nc = build_nc()
    res = None
    last_err = None
    for attempt in range(3):
        try:
            res = run_bass_kernel_spmd(nc, in_maps, core_ids=list(range(NCORES)))
            break
        except Exception as e:  # transient NRT_EXEC_UNIT_UNRECOVERABLE on first exec
            last_err = e
    if res is None:
        raise last_err
    y = np.concatenate(
        [res.results[c]["y"] for c in range(NCORES)], axis=0
    ).astype(np.float32)
    return y


if __name__ == "__main__":
    xs = np.random.randn(B, T, C).astype(np.float32)
    print("built", build_nc())



# revision 7
# speedup vs baseline: 930.3104x; 930.3104x over previous
"""Soft-MoE forward on 8 TRN2 NeuronCores.

Strategy: pure data-parallel over the batch axis (B=16 -> 2 batches/core).
Each core runs router + dispatch + all-expert GLU-MLP + combine for its two
batches; expert weights are replicated (streamed from HBM in bf16). No
cross-core communication: the expert MLP is row-independent, and each batch
contributes CAP=128 rows per expert.

All matmuls bf16 (fp8 DoubleRow measures 2.04x PE throughput on HW but its
~3%-per-tensor quantization noise blows the 2e-2 gate; compensated fp8
needs 3 products = 1.5x bf16 cycles, a net loss). Optimizations vs the
original baseline:
  - M phase software-pipelined one expert deep: proj(e-1) issues after
    ffn(e), so the PE never waits on the h DMA-transpose.
  - gg/hh matmuls share each stationary eit tile (gate+fc back to back),
    and proj uses 512-wide moving tiles (512 rows per weight switch).
  - Combine accumulates both 512-column halves under one ptr weight load
    (1024 rows per weight switch).
  - Host-packed x^T tiles and tile-major PT staging keep every DMA's
    contiguous run >= 512B (sub-512B runs pay 2x on the DMA bus).
  - Deeper tile-pool rings on weight/eit/h tiles so weight streaming
    prefetches across experts.
Softmax statistics in fp32; exp() skips max-subtraction (logits are
~N(0, 0.64^2), well within range).
"""

import numpy as np
import ml_dtypes

import concourse.bass as bass
import concourse.tile as tile
from concourse import mybir

B, T, C, E, H = 16, 2048, 1024, 16, 2048
CAP = T // E  # 128
S = E * CAP  # 2048 slots
P = 128  # partitions
NCORES = 8
BL = B // NCORES  # 2 batches per core

FP32 = mybir.dt.float32
BF16 = mybir.dt.bfloat16
AX = mybir.AluOpType


def _split_multi_waits(nc):
    """This walrus build accepts only ONE sync wait per instruction; Tile's
    wait-assignment can emit several. Move extra waits onto single-wait nops
    inserted just before the instruction on the same engine (same-engine
    program order makes this semantically identical)."""
    import bass_rust

    nid = 0
    for f in nc.m.functions:
        for bb in f.blocks:
            out = []
            changed = False
            for inst in bb.instructions:
                si = inst.sync_info
                waits = list(si.on_wait) if si and si.on_wait else []
                if len(waits) > 1:
                    changed = True
                    for w in waits[:-1]:
                        nop = mybir.InstNoOp(name=f"TW-{nid}", ins=[], outs=[])
                        nid += 1
                        nop.engine = inst.engine
                        nop.sync_info = bass_rust.SyncInfo(
                            on_wait=[w], on_update=[]
                        )
                        out.append(nop)
                    si.on_wait = waits[-1:]
                out.append(inst)
            if changed:
                bb.instructions = out
    return nc


def build_nc(loops=BL):
    assert loops % BL == 0
    nc = bass.Bass(trn_type="TRN2")

    xb = nc.dram_tensor("xb", [BL, T, C], BF16, kind="ExternalInput")
    # packed x^T tiles: xbtp[b, t, p, c*128+i] = x[b, t*128+i, c*128+p]
    xbtp = nc.dram_tensor("xbtp", [BL, T // P, P, C], BF16, kind="ExternalInput")
    wrt = nc.dram_tensor("wrt", [C, S], BF16, kind="ExternalInput")
    wg = nc.dram_tensor("wg", [E, C, H], BF16, kind="ExternalInput")
    wf = nc.dram_tensor("wf", [E, C, H], BF16, kind="ExternalInput")
    wp = nc.dram_tensor("wp", [E, H, C], BF16, kind="ExternalInput")
    y = nc.dram_tensor("y", [BL, T, C], FP32, kind="ExternalOutput")

    Tt = T // P  # 16 T tiles
    Ct = C // P  # 8 C tiles
    St = S // P  # 16 slot tiles (= experts)
    Ht = H // P  # 16 H tiles

    with tile.TileContext(nc) as tc:
        with (
            tc.tile_pool(name="dram", bufs=2, space="DRAM") as dpool,
            tc.tile_pool(name="const", bufs=1) as cpool,
            tc.tile_pool(name="io3", bufs=3) as p3,
            tc.tile_pool(name="io2", bufs=2) as p2,
            tc.tile_pool(name="stat", bufs=4) as sp,
            tc.tile_pool(name="psum", bufs=8, space="PSUM") as pp,
        ):
            for rep in range(loops // BL):
                # router weights resident: [128, Ct, S]; tag shared with eo
                # (wrt is dead by the time eo tiles are written)
                wrt_sb = cpool.tile([P, Ct, S], BF16, tag="big", bufs=2, name="wrt_sb")
                nc.sync.dma_start(
                    wrt_sb[:], wrt.rearrange("(c p) s -> p c s", p=P)
                )
                p_drams, pt_drams, eit_drams, rzcs, rzds = {}, {}, {}, {}, {}
                for b in range(BL):
                    p_dram = dpool.tile([T, S], BF16, tag="p_dram", name=f"pd{b}")
                    # tile-major PT staging: pt2[t][p][s*128+i] = PT[s*128+p, t*128+i]
                    pt_dram = dpool.tile([Tt, P, S], BF16, tag="pt_dram", name=f"ptd{b}")
                    eit_dram = dpool.tile([C, S], BF16, tag="eit_dram", name=f"eitd{b}")
                    rzc = p2.tile([P, Tt], FP32, tag="rzc", name=f"rzc{b}")
                    zdall = p2.tile([P, Tt, St], FP32, tag="zdall", name=f"zdall{b}")
                    rzd = p2.tile([P, St], FP32, tag="rzd", name=f"rzd{b}")
                    p_drams[b], pt_drams[b], eit_drams[b] = p_dram, pt_dram, eit_dram
                    rzcs[b], rzds[b] = rzc, rzd

                    # ---- Phase R: router logits g = x @ wr^T, P = exp(g) ----
                    for t in range(Tt):
                        xbt_t = p3.tile([P, Ct, P], BF16, tag="xbt_t")
                        nc.sync.dma_start(
                            xbt_t[:],
                            xbtp[b, t].rearrange("p (c i) -> p c i", c=Ct),
                        )
                        gps = [pp.tile([P, 512], FP32, tag="ps", name=f"gps{n}") for n in range(4)]
                        for c in range(Ct):
                            for n in range(4):
                                nc.tensor.matmul(
                                    gps[n][:],
                                    xbt_t[:, c, :],
                                    wrt_sb[:, c, n * 512 : (n + 1) * 512],
                                    start=(c == 0),
                                    stop=(c == Ct - 1),
                                )
                        pch = p3.tile([P, S], BF16, tag="pch")
                        zc4 = sp.tile([P, 4], FP32, tag="zc4")
                        for n in range(4):
                            nc.scalar.activation(
                                pch[:, n * 512 : (n + 1) * 512],
                                gps[n][:],
                                mybir.ActivationFunctionType.Exp,
                                accum_out=zc4[:, n : n + 1],
                            )
                        zc1 = sp.tile([P, 1], FP32, tag="zc1")
                        nc.vector.tensor_reduce(zc1[:], zc4[:], mybir.AxisListType.X, AX.add)
                        nc.vector.reciprocal(rzc[:, t : t + 1], zc1[:])
                        nc.gpsimd.dma_start(
                            p_dram[t * P : (t + 1) * P, :], pch[:]
                        )

                    # ---- Phase T: transpose P -> PT (DMA xbar), Zd partials ----
                    for t in range(Tt):
                        ptt = p3.tile([P, St, P], BF16, tag="ptt")
                        nc.scalar.dma_start_transpose(
                            ptt[:], p_dram[t * P : (t + 1) * P, :]
                        )
                        nc.vector.tensor_reduce(
                            zdall[:, t, :], ptt[:], mybir.AxisListType.X, AX.add
                        )
                        nc.gpsimd.dma_start(
                            pt_dram[t].rearrange("p (s i) -> p s i", s=St), ptt[:]
                        )
                    zd = sp.tile([P, St], FP32, tag="zd")
                    nc.vector.tensor_reduce(
                        zd[:], zdall.rearrange("p t s -> p s t"), mybir.AxisListType.X, AX.add
                    )
                    nc.vector.reciprocal(rzd[:], zd[:])

                    # ---- Phase D: dispatch eiT = x^T @ P (unnormalized) ----
                    for nh in range(2):
                        for mg in range(2):
                            dps = [pp.tile([P, 512], FP32, tag="ps", name=f"dps{i}") for i in range(8)]
                            for k in range(Tt):
                                xk = p3.tile([P, 512], BF16, tag="xk")
                                nc.sync.dma_start(
                                    xk[:],
                                    xb[b].rearrange("(k p) c -> p k c", p=P)[
                                        :, k, mg * 512 : (mg + 1) * 512
                                    ],
                                )
                                pk = p3.tile([P, 1024], BF16, tag="pk")
                                nc.sync.dma_start(
                                    pk[:],
                                    p_dram[
                                        k * P : (k + 1) * P,
                                        nh * 1024 : (nh + 1) * 1024,
                                    ],
                                )
                                for m4 in range(4):
                                    for n2 in range(2):
                                        nc.tensor.matmul(
                                            dps[m4 * 2 + n2][:],
                                            xk[:, m4 * P : (m4 + 1) * P],
                                            pk[:, n2 * 512 : (n2 + 1) * 512],
                                            start=(k == 0),
                                            stop=(k == Tt - 1),
                                        )
                            for i in range(8):
                                m = mg * 4 + i // 2
                                n = nh * 2 + i % 2
                                et = p3.tile([P, 512], BF16, tag="et")
                                nc.scalar.copy(et[:], dps[i][:])
                                nc.gpsimd.dma_start(
                                    eit_dram[
                                        m * P : (m + 1) * P, n * 512 : (n + 1) * 512
                                    ],
                                    et[:],
                                )

                # ---- Phase M: per-expert GLU MLP, 1-expert-deep skew ----
                eos = {
                    b: cpool.tile([P, St, C], BF16, tag="big", bufs=2, name=f"eo{b}")
                    for b in range(BL)
                }
                hts = {}  # (e, b) -> transposed h, alive ffn(e)..proj(e)

                def ffn(e):
                    eites = {}
                    for b in range(BL):
                        eit_e = p2.tile([P, Ct, P], BF16, tag="eit_e", bufs=4, name=f"eite{b}")
                        nc.sync.dma_start(
                            eit_e[:],
                            eit_drams[b].rearrange("(c p) s -> p c s", p=P)[
                                :, :, e * P : (e + 1) * P
                            ],
                        )
                        eites[b] = eit_e
                    hs = {
                        b: p2.tile([P, H], BF16, tag="h_sb", bufs=2, name=f"h{b}")
                        for b in range(BL)
                    }
                    for hc in range(4):
                        wgc = p2.tile([P, Ct, 512], BF16, tag="wgc", bufs=2)
                        nc.sync.dma_start(
                            wgc[:],
                            wg[e].rearrange("(c p) h -> p c h", p=P)[
                                :, :, hc * 512 : (hc + 1) * 512
                            ],
                        )
                        wfcc = p2.tile([P, Ct, 512], BF16, tag="wfcc", bufs=2)
                        nc.sync.dma_start(
                            wfcc[:],
                            wf[e].rearrange("(c p) h -> p c h", p=P)[
                                :, :, hc * 512 : (hc + 1) * 512
                            ],
                        )
                        for b in range(BL):
                            gg = pp.tile([P, 512], FP32, tag="ps", name=f"gg{b}")
                            hh = pp.tile([P, 512], FP32, tag="ps", name=f"hh{b}")
                            # share each stationary eit tile: gate+fc per c
                            for c in range(Ct):
                                nc.tensor.matmul(
                                    gg[:], eites[b][:, c, :], wgc[:, c, :],
                                    start=(c == 0), stop=(c == Ct - 1),
                                )
                                nc.tensor.matmul(
                                    hh[:], eites[b][:, c, :], wfcc[:, c, :],
                                    start=(c == 0), stop=(c == Ct - 1),
                                )
                            sg = p3.tile([P, 512], BF16, tag="sg", name=f"sg{b}")
                            nc.scalar.activation(
                                sg[:], gg[:],
                                mybir.ActivationFunctionType.Silu,
                                scale=rzds[b][:, e : e + 1],
                            )
                            nc.vector.scalar_tensor_tensor(
                                hs[b][:, hc * 512 : (hc + 1) * 512],
                                hh[:], rzds[b][:, e : e + 1], sg[:],
                                AX.mult, AX.mult,
                            )
                    for b in range(BL):
                        ht = p2.tile([P, Ht, P], BF16, tag="ht", bufs=4, name=f"ht{b}")
                        nc.scalar.dma_start_transpose(ht[:], hs[b][:])
                        hts[(e, b)] = ht

                def proj(e):
                    for cc2 in range(2):
                        # wp weights in two k-half tiles (SBUF pressure)
                        wpcs = []
                        for kh in range(2):
                            wpc = p2.tile([P, Ht // 2, 512], BF16, tag="wpc", bufs=2)
                            nc.sync.dma_start(
                                wpc[:],
                                wp[e].rearrange("(k p) c -> p k c", p=P)[
                                    :, kh * (Ht // 2) : (kh + 1) * (Ht // 2),
                                    cc2 * 512 : (cc2 + 1) * 512
                                ],
                            )
                            wpcs.append(wpc)
                        for b in range(BL):
                            eop = pp.tile([P, 512], FP32, tag="ps", name=f"eop{b}")
                            for k in range(Ht):
                                nc.tensor.matmul(
                                    eop[:], hts[(e, b)][:, k, :],
                                    wpcs[k // (Ht // 2)][:, k % (Ht // 2), :],
                                    start=(k == 0), stop=(k == Ht - 1),
                                )
                            nc.scalar.copy(
                                eos[b][:, e, cc2 * 512 : (cc2 + 1) * 512], eop[:]
                            )
                    for b in range(BL):
                        del hts[(e, b)]

                for e in range(E):
                    ffn(e)
                    if e >= 1:
                        proj(e - 1)
                proj(E - 1)

                # ---- Phase C: combine y = (P @ eo) * rzc ----
                for b in range(BL):
                    for t in range(Tt):
                        ptr = p2.tile([P, St, P], BF16, tag="ptr")
                        nc.sync.dma_start(
                            ptr[:],
                            pt_drams[b][t].rearrange("p (s i) -> p s i", s=St),
                        )
                        yps = [pp.tile([P, 512], FP32, tag="ps", name=f"yps{cc}") for cc in range(2)]
                        # both 512-col halves under one ptr weight load
                        for e in range(St):
                            for cc in range(2):
                                nc.tensor.matmul(
                                    yps[cc][:],
                                    ptr[:, e, :],
                                    eos[b][:, e, cc * 512 : (cc + 1) * 512],
                                    start=(e == 0),
                                    stop=(e == St - 1),
                                )
                        for cc in range(2):
                            ysb = p3.tile([P, 512], FP32, tag="ysb")
                            nc.vector.tensor_scalar_mul(
                                ysb[:], yps[cc][:], rzcs[b][:, t : t + 1]
                            )
                            nc.gpsimd.dma_start(
                                y[b, t * P : (t + 1) * P, cc * 512 : (cc + 1) * 512],
                                ysb[:],
                            )
    _split_multi_waits(nc)
    return nc


def kernel(x, w_router_gate, w_fc, w_gate, w_proj):
    bf16 = ml_dtypes.bfloat16
    wrt_np = np.ascontiguousarray(
        w_router_gate.reshape(S, C).T
    ).astype(bf16)
    wg_np = w_gate.astype(bf16)
    wf_np = w_fc.astype(bf16)
    wp_np = w_proj.astype(bf16)

    in_maps = []
    for c in range(NCORES):
        xc = x[c * BL : (c + 1) * BL]  # [BL, T, C] fp32
        xb_np = xc.astype(bf16)
        # xbtp[b, t, p, c*128+i] = x[b, t*128+i, c*128+p]
        xbtp_np = np.ascontiguousarray(
            xb_np.reshape(BL, T // P, P, C // P, P).transpose(0, 1, 4, 3, 2)
        ).reshape(BL, T // P, P, C)
        in_maps.append(
            {
                "xb": xb_np,
                "xbtp": xbtp_np,
                "wrt": wrt_np,
                "wg": wg_np,
                "wf": wf_np,
                "wp": wp_np,
            }
        )

    from concourse.bass_utils import run_bass_kernel_spmd

    nc = build_nc()
    res = None
    last_err = None
    for attempt in range(3):
        try:
            res = run_bass_kernel_spmd(nc, in_maps, core_ids=list(range(NCORES)))
            break
        except Exception as e:  # transient NRT_EXEC_UNIT_UNRECOVERABLE on first exec
            last_err = e
    if res is None:
        raise last_err
    y = np.concatenate(
        [res.results[c]["y"] for c in range(NCORES)], axis=0
    ).astype(np.float32)
    return y


if __name__ == "__main__":
    xs = np.random.randn(B, T, C).astype(np.float32)
    print("built", build_nc())


# revision 8
# speedup vs baseline: 1011.9209x; 1.0877x over previous
"""Soft-MoE forward on 8 TRN2 NeuronCores.

Strategy: pure data-parallel over the batch axis (B=16 -> 2 batches/core).
Each core runs router + dispatch + all-expert GLU-MLP + combine for its two
batches; expert weights are replicated (streamed from HBM in bf16). No
cross-core communication: the expert MLP is row-independent, and each batch
contributes CAP=128 rows per expert.

All matmuls bf16 (fp8 DoubleRow measures 2.04x PE throughput on HW but its
~3%-per-tensor quantization noise blows the 2e-2 gate; compensated fp8
needs 3 products = 1.5x bf16 cycles, a net loss). Optimizations vs the
original baseline:
  - M phase software-pipelined one expert deep: proj(e-1) issues after
    ffn(e), so the PE never waits on the h DMA-transpose.
  - gg/hh matmuls share each stationary eit tile (gate+fc back to back),
    and proj uses 512-wide moving tiles (512 rows per weight switch).
  - Combine accumulates both 512-column halves under one ptr weight load
    (1024 rows per weight switch).
  - Host-packed x^T tiles and tile-major PT staging keep every DMA's
    contiguous run >= 512B (sub-512B runs pay 2x on the DMA bus).
  - Deeper tile-pool rings on weight/eit/h tiles so weight streaming
    prefetches across experts.
Softmax statistics in fp32; exp() skips max-subtraction (logits are
~N(0, 0.64^2), well within range).
"""

import numpy as np
import ml_dtypes

import concourse.bass as bass
import concourse.tile as tile
from concourse import mybir

B, T, C, E, H = 16, 2048, 1024, 16, 2048
CAP = T // E  # 128
S = E * CAP  # 2048 slots
P = 128  # partitions
NCORES = 8
BL = B // NCORES  # 2 batches per core

FP32 = mybir.dt.float32
BF16 = mybir.dt.bfloat16
AX = mybir.AluOpType


def _split_multi_waits(nc):
    """This walrus build accepts only ONE sync wait per instruction; Tile's
    wait-assignment can emit several. Move extra waits onto single-wait nops
    inserted just before the instruction on the same engine (same-engine
    program order makes this semantically identical)."""
    import bass_rust

    nid = 0
    for f in nc.m.functions:
        for bb in f.blocks:
            out = []
            changed = False
            for inst in bb.instructions:
                si = inst.sync_info
                waits = list(si.on_wait) if si and si.on_wait else []
                if len(waits) > 1:
                    changed = True
                    for w in waits[:-1]:
                        nop = mybir.InstNoOp(name=f"TW-{nid}", ins=[], outs=[])
                        nid += 1
                        nop.engine = inst.engine
                        nop.sync_info = bass_rust.SyncInfo(
                            on_wait=[w], on_update=[]
                        )
                        out.append(nop)
                    si.on_wait = waits[-1:]
                out.append(inst)
            if changed:
                bb.instructions = out
    return nc


def build_nc(loops=BL):
    assert loops % BL == 0
    nc = bass.Bass(trn_type="TRN2")

    xb = nc.dram_tensor("xb", [BL, T, C], BF16, kind="ExternalInput")
    # packed x^T tiles: xbtp[b, t, p, c*128+i] = x[b, t*128+i, c*128+p]
    xbtp = nc.dram_tensor("xbtp", [BL, T // P, P, C], BF16, kind="ExternalInput")
    wrt = nc.dram_tensor("wrt", [C, S], BF16, kind="ExternalInput")
    wg = nc.dram_tensor("wg", [E, C, H], BF16, kind="ExternalInput")
    wf = nc.dram_tensor("wf", [E, C, H], BF16, kind="ExternalInput")
    wp = nc.dram_tensor("wp", [E, H, C], BF16, kind="ExternalInput")
    y = nc.dram_tensor("y", [BL, T, C], FP32, kind="ExternalOutput")

    Tt = T // P  # 16 T tiles
    Ct = C // P  # 8 C tiles
    St = S // P  # 16 slot tiles (= experts)
    Ht = H // P  # 16 H tiles

    with tile.TileContext(nc) as tc:
        with (
            tc.tile_pool(name="dram", bufs=2, space="DRAM") as dpool,
            tc.tile_pool(name="const", bufs=1) as cpool,
            tc.tile_pool(name="io3", bufs=3) as p3,
            tc.tile_pool(name="io2", bufs=2) as p2,
            tc.tile_pool(name="stat", bufs=4) as sp,
            tc.tile_pool(name="psum", bufs=8, space="PSUM") as pp,
        ):
            for rep in range(loops // BL):
                # router weights resident: [128, Ct, S]; tag shared with eo
                # (wrt is dead by the time eo tiles are written).  Loaded in
                # Ct chunk DMAs so the first router matmul only waits on the
                # first chunk.
                wrt_sb = cpool.tile([P, Ct, S], BF16, tag="big", bufs=2, name="wrt_sb")
                wrt_view = wrt.rearrange("(c p) s -> p c s", p=P)
                for c in range(Ct):
                    nc.sync.dma_start(
                        wrt_sb[:, c : c + 1, :], wrt_view[:, c : c + 1, :]
                    )
                p_drams, pt_drams, eit_drams, rzcs, rzds = {}, {}, {}, {}, {}
                for b in range(BL):
                    p_dram = dpool.tile([T, S], BF16, tag="p_dram", name=f"pd{b}")
                    # tile-major PT staging: pt2[t][p][s*128+i] = PT[s*128+p, t*128+i]
                    pt_dram = dpool.tile([Tt, P, S], BF16, tag="pt_dram", name=f"ptd{b}")
                    eit_dram = dpool.tile([C, S], BF16, tag="eit_dram", name=f"eitd{b}")
                    rzc = p2.tile([P, Tt], FP32, tag="rzc", name=f"rzc{b}")
                    zdall = p2.tile([P, Tt, St], FP32, tag="zdall", name=f"zdall{b}")
                    rzd = p2.tile([P, St], FP32, tag="rzd", name=f"rzd{b}")
                    p_drams[b], pt_drams[b], eit_drams[b] = p_dram, pt_dram, eit_dram
                    rzcs[b], rzds[b] = rzc, rzd

                    # ---- Phase R: router logits g = x @ wr^T, P = exp(g) ----
                    for t in range(Tt):
                        xbt_t = p3.tile([P, Ct, P], BF16, tag="xbt_t")
                        nc.sync.dma_start(
                            xbt_t[:],
                            xbtp[b, t].rearrange("p (c i) -> p c i", c=Ct),
                        )
                        gps = [pp.tile([P, 512], FP32, tag="ps", name=f"gps{n}") for n in range(4)]
                        for c in range(Ct):
                            for n in range(4):
                                nc.tensor.matmul(
                                    gps[n][:],
                                    xbt_t[:, c, :],
                                    wrt_sb[:, c, n * 512 : (n + 1) * 512],
                                    start=(c == 0),
                                    stop=(c == Ct - 1),
                                )
                        pch = p3.tile([P, S], BF16, tag="pch")
                        zc4 = sp.tile([P, 4], FP32, tag="zc4")
                        for n in range(4):
                            nc.scalar.activation(
                                pch[:, n * 512 : (n + 1) * 512],
                                gps[n][:],
                                mybir.ActivationFunctionType.Exp,
                                accum_out=zc4[:, n : n + 1],
                            )
                        zc1 = sp.tile([P, 1], FP32, tag="zc1")
                        nc.vector.tensor_reduce(zc1[:], zc4[:], mybir.AxisListType.X, AX.add)
                        nc.vector.reciprocal(rzc[:, t : t + 1], zc1[:])
                        nc.gpsimd.dma_start(
                            p_dram[t * P : (t + 1) * P, :], pch[:]
                        )

                    # ---- Phase T: transpose P -> PT (DMA xbar), Zd partials ----
                    for t in range(Tt):
                        ptt = p3.tile([P, St, P], BF16, tag="ptt")
                        nc.scalar.dma_start_transpose(
                            ptt[:], p_dram[t * P : (t + 1) * P, :]
                        )
                        nc.vector.tensor_reduce(
                            zdall[:, t, :], ptt[:], mybir.AxisListType.X, AX.add
                        )
                        nc.gpsimd.dma_start(
                            pt_dram[t].rearrange("p (s i) -> p s i", s=St), ptt[:]
                        )
                    zd = sp.tile([P, St], FP32, tag="zd")
                    nc.vector.tensor_reduce(
                        zd[:], zdall.rearrange("p t s -> p s t"), mybir.AxisListType.X, AX.add
                    )
                    nc.vector.reciprocal(rzd[:], zd[:])

                    # ---- Phase D: dispatch eiT = x^T @ P (unnormalized) ----
                    for nh in range(2):
                        for mg in range(2):
                            dps = [pp.tile([P, 512], FP32, tag="ps", name=f"dps{i}") for i in range(8)]
                            for k in range(Tt):
                                xk = p3.tile([P, 512], BF16, tag="xk")
                                nc.sync.dma_start(
                                    xk[:],
                                    xb[b].rearrange("(k p) c -> p k c", p=P)[
                                        :, k, mg * 512 : (mg + 1) * 512
                                    ],
                                )
                                pk = p3.tile([P, 1024], BF16, tag="pk")
                                nc.sync.dma_start(
                                    pk[:],
                                    p_dram[
                                        k * P : (k + 1) * P,
                                        nh * 1024 : (nh + 1) * 1024,
                                    ],
                                )
                                for m4 in range(4):
                                    for n2 in range(2):
                                        nc.tensor.matmul(
                                            dps[m4 * 2 + n2][:],
                                            xk[:, m4 * P : (m4 + 1) * P],
                                            pk[:, n2 * 512 : (n2 + 1) * 512],
                                            start=(k == 0),
                                            stop=(k == Tt - 1),
                                        )
                            for i in range(8):
                                m = mg * 4 + i // 2
                                n = nh * 2 + i % 2
                                et = p3.tile([P, 512], BF16, tag="et")
                                nc.scalar.copy(et[:], dps[i][:])
                                nc.gpsimd.dma_start(
                                    eit_dram[
                                        m * P : (m + 1) * P, n * 512 : (n + 1) * 512
                                    ],
                                    et[:],
                                )

                # ---- Phase M: per-expert GLU MLP, 1-expert-deep skew ----
                eos = {
                    b: cpool.tile([P, St, C], BF16, tag="big", bufs=2, name=f"eo{b}")
                    for b in range(BL)
                }
                hts = {}  # (e, b) -> transposed h, alive ffn(e)..proj(e)

                def ffn(e):
                    eites = {}
                    for b in range(BL):
                        eit_e = p2.tile([P, Ct, P], BF16, tag="eit_e", bufs=4, name=f"eite{b}")
                        nc.sync.dma_start(
                            eit_e[:],
                            eit_drams[b].rearrange("(c p) s -> p c s", p=P)[
                                :, :, e * P : (e + 1) * P
                            ],
                        )
                        eites[b] = eit_e
                    hs = {
                        b: p2.tile([P, H], BF16, tag="h_sb", bufs=2, name=f"h{b}")
                        for b in range(BL)
                    }
                    for hc in range(4):
                        wgc = p2.tile([P, Ct, 512], BF16, tag="wgc", bufs=2)
                        nc.sync.dma_start(
                            wgc[:],
                            wg[e].rearrange("(c p) h -> p c h", p=P)[
                                :, :, hc * 512 : (hc + 1) * 512
                            ],
                        )
                        wfcc = p2.tile([P, Ct, 512], BF16, tag="wfcc", bufs=2)
                        nc.sync.dma_start(
                            wfcc[:],
                            wf[e].rearrange("(c p) h -> p c h", p=P)[
                                :, :, hc * 512 : (hc + 1) * 512
                            ],
                        )
                        for b in range(BL):
                            gg = pp.tile([P, 512], FP32, tag="ps", name=f"gg{b}")
                            hh = pp.tile([P, 512], FP32, tag="ps", name=f"hh{b}")
                            # share each stationary eit tile: gate+fc per c
                            for c in range(Ct):
                                nc.tensor.matmul(
                                    gg[:], eites[b][:, c, :], wgc[:, c, :],
                                    start=(c == 0), stop=(c == Ct - 1),
                                )
                                nc.tensor.matmul(
                                    hh[:], eites[b][:, c, :], wfcc[:, c, :],
                                    start=(c == 0), stop=(c == Ct - 1),
                                )
                            sg = p3.tile([P, 512], BF16, tag="sg", name=f"sg{b}")
                            nc.scalar.activation(
                                sg[:], gg[:],
                                mybir.ActivationFunctionType.Silu,
                                scale=rzds[b][:, e : e + 1],
                            )
                            nc.vector.scalar_tensor_tensor(
                                hs[b][:, hc * 512 : (hc + 1) * 512],
                                hh[:], rzds[b][:, e : e + 1], sg[:],
                                AX.mult, AX.mult,
                            )
                    for b in range(BL):
                        ht = p2.tile([P, Ht, P], BF16, tag="ht", bufs=4, name=f"ht{b}")
                        nc.scalar.dma_start_transpose(ht[:], hs[b][:])
                        hts[(e, b)] = ht

                def proj(e):
                    for cc2 in range(2):
                        # wp weights in two k-half tiles (SBUF pressure)
                        wpcs = []
                        for kh in range(2):
                            wpc = p2.tile([P, Ht // 2, 512], BF16, tag="wpc", bufs=2)
                            nc.sync.dma_start(
                                wpc[:],
                                wp[e].rearrange("(k p) c -> p k c", p=P)[
                                    :, kh * (Ht // 2) : (kh + 1) * (Ht // 2),
                                    cc2 * 512 : (cc2 + 1) * 512
                                ],
                            )
                            wpcs.append(wpc)
                        for b in range(BL):
                            eop = pp.tile([P, 512], FP32, tag="ps", name=f"eop{b}")
                            for k in range(Ht):
                                nc.tensor.matmul(
                                    eop[:], hts[(e, b)][:, k, :],
                                    wpcs[k // (Ht // 2)][:, k % (Ht // 2), :],
                                    start=(k == 0), stop=(k == Ht - 1),
                                )
                            nc.scalar.copy(
                                eos[b][:, e, cc2 * 512 : (cc2 + 1) * 512], eop[:]
                            )
                    for b in range(BL):
                        del hts[(e, b)]

                for e in range(E):
                    ffn(e)
                    if e >= 1:
                        proj(e - 1)
                proj(E - 1)

                # ---- Phase C: combine y = (P @ eo) * rzc ----
                for b in range(BL):
                    for t in range(Tt):
                        ptr = p2.tile([P, St, P], BF16, tag="ptr")
                        nc.sync.dma_start(
                            ptr[:],
                            pt_drams[b][t].rearrange("p (s i) -> p s i", s=St),
                        )
                        yps = [pp.tile([P, 512], FP32, tag="ps", name=f"yps{cc}") for cc in range(2)]
                        # both 512-col halves under one ptr weight load
                        for e in range(St):
                            for cc in range(2):
                                nc.tensor.matmul(
                                    yps[cc][:],
                                    ptr[:, e, :],
                                    eos[b][:, e, cc * 512 : (cc + 1) * 512],
                                    start=(e == 0),
                                    stop=(e == St - 1),
                                )
                        for cc in range(2):
                            ysb = p3.tile([P, 512], FP32, tag="ysb")
                            nc.vector.tensor_scalar_mul(
                                ysb[:], yps[cc][:], rzcs[b][:, t : t + 1]
                            )
                            nc.gpsimd.dma_start(
                                y[b, t * P : (t + 1) * P, cc * 512 : (cc + 1) * 512],
                                ysb[:],
                            )
    _split_multi_waits(nc)
    return nc


def kernel(x, w_router_gate, w_fc, w_gate, w_proj):
    bf16 = ml_dtypes.bfloat16
    wrt_np = np.ascontiguousarray(
        w_router_gate.reshape(S, C).T
    ).astype(bf16)
    wg_np = w_gate.astype(bf16)
    wf_np = w_fc.astype(bf16)
    wp_np = w_proj.astype(bf16)

    in_maps = []
    for c in range(NCORES):
        xc = x[c * BL : (c + 1) * BL]  # [BL, T, C] fp32
        xb_np = xc.astype(bf16)
        # xbtp[b, t, p, c*128+i] = x[b, t*128+i, c*128+p]
        xbtp_np = np.ascontiguousarray(
            xb_np.reshape(BL, T // P, P, C // P, P).transpose(0, 1, 4, 3, 2)
        ).reshape(BL, T // P, P, C)
        in_maps.append(
            {
                "xb": xb_np,
                "xbtp": xbtp_np,
                "wrt": wrt_np,
                "wg": wg_np,
                "wf": wf_np,
                "wp": wp_np,
            }
        )

    from concourse.bass_utils import run_bass_kernel_spmd

    nc = build_nc()
    res = None
    last_err = None
    for attempt in range(3):
        try:
            res = run_bass_kernel_spmd(nc, in_maps, core_ids=list(range(NCORES)))
            break
        except Exception as e:  # transient NRT_EXEC_UNIT_UNRECOVERABLE on first exec
            last_err = e
    if res is None:
        raise last_err
    y = np.concatenate(
        [res.results[c]["y"] for c in range(NCORES)], axis=0
    ).astype(np.float32)
    return y


if __name__ == "__main__":
    xs = np.random.randn(B, T, C).astype(np.float32)
    print("built", build_nc())


# revision 9
# speedup vs baseline: 1059.8448x; 1.0474x over previous
"""Soft-MoE forward on 8 TRN2 NeuronCores.

Strategy: pure data-parallel over the batch axis (B=16 -> 2 batches/core).
Each core runs router + dispatch + all-expert GLU-MLP + combine for its two
batches; expert weights are replicated (streamed from HBM in bf16). No
cross-core communication: the expert MLP is row-independent, and each batch
contributes CAP=128 rows per expert.

All matmuls bf16 (fp8 DoubleRow measures 2.04x PE throughput on HW but its
~3%-per-tensor quantization noise blows the 2e-2 gate; compensated fp8
needs 3 products = 1.5x bf16 cycles, a net loss). Optimizations vs the
original baseline:
  - M phase software-pipelined one expert deep: proj(e-1) issues after
    ffn(e), so the PE never waits on the h DMA-transpose.
  - gg/hh matmuls share each stationary eit tile (gate+fc back to back),
    and proj uses 512-wide moving tiles (512 rows per weight switch).
  - Combine accumulates both 512-column halves under one ptr weight load
    (1024 rows per weight switch).
  - Host-packed x^T tiles and tile-major PT staging keep every DMA's
    contiguous run >= 512B (sub-512B runs pay 2x on the DMA bus).
  - Deeper tile-pool rings on weight/eit/h tiles so weight streaming
    prefetches across experts.
Softmax statistics in fp32; exp() skips max-subtraction (logits are
~N(0, 0.64^2), well within range).
"""

import numpy as np
import ml_dtypes

import concourse.bass as bass
import concourse.tile as tile
from concourse import mybir

B, T, C, E, H = 16, 2048, 1024, 16, 2048
CAP = T // E  # 128
S = E * CAP  # 2048 slots
P = 128  # partitions
NCORES = 8
BL = B // NCORES  # 2 batches per core

FP32 = mybir.dt.float32
BF16 = mybir.dt.bfloat16
AX = mybir.AluOpType


def _split_multi_waits(nc):
    """This walrus build accepts only ONE sync wait per instruction; Tile's
    wait-assignment can emit several. Move extra waits onto single-wait nops
    inserted just before the instruction on the same engine (same-engine
    program order makes this semantically identical)."""
    import bass_rust

    nid = 0
    for f in nc.m.functions:
        for bb in f.blocks:
            out = []
            changed = False
            for inst in bb.instructions:
                si = inst.sync_info
                waits = list(si.on_wait) if si and si.on_wait else []
                if len(waits) > 1:
                    changed = True
                    for w in waits[:-1]:
                        nop = mybir.InstNoOp(name=f"TW-{nid}", ins=[], outs=[])
                        nid += 1
                        nop.engine = inst.engine
                        nop.sync_info = bass_rust.SyncInfo(
                            on_wait=[w], on_update=[]
                        )
                        out.append(nop)
                    si.on_wait = waits[-1:]
                out.append(inst)
            if changed:
                bb.instructions = out
    return nc


def build_nc(loops=BL):
    assert loops % BL == 0
    nc = bass.Bass(trn_type="TRN2")

    xb = nc.dram_tensor("xb", [BL, T, C], BF16, kind="ExternalInput")
    # packed x^T tiles: xbtp[b, t, p, c*128+i] = x[b, t*128+i, c*128+p]
    xbtp = nc.dram_tensor("xbtp", [BL, T // P, P, C], BF16, kind="ExternalInput")
    wrt = nc.dram_tensor("wrt", [C, S], BF16, kind="ExternalInput")
    wg = nc.dram_tensor("wg", [E, C, H], BF16, kind="ExternalInput")
    wf = nc.dram_tensor("wf", [E, C, H], BF16, kind="ExternalInput")
    wp = nc.dram_tensor("wp", [E, H, C], BF16, kind="ExternalInput")
    y = nc.dram_tensor("y", [BL, T, C], FP32, kind="ExternalOutput")

    Tt = T // P  # 16 T tiles
    Ct = C // P  # 8 C tiles
    St = S // P  # 16 slot tiles (= experts)
    Ht = H // P  # 16 H tiles

    with tile.TileContext(nc) as tc:
        with (
            tc.tile_pool(name="dram", bufs=2, space="DRAM") as dpool,
            tc.tile_pool(name="const", bufs=1) as cpool,
            tc.tile_pool(name="io3", bufs=3) as p3,
            tc.tile_pool(name="io2", bufs=2) as p2,
            tc.tile_pool(name="stat", bufs=4) as sp,
            tc.tile_pool(name="psum", bufs=8, space="PSUM") as pp,
        ):
            for rep in range(loops // BL):
                # router weights resident: [128, Ct, S]; tag shared with eo
                # (wrt is dead by the time eo tiles are written).  Loaded in
                # Ct chunk DMAs so the first router matmul only waits on the
                # first chunk.
                wrt_sb = cpool.tile([P, Ct, S], BF16, tag="big", bufs=2, name="wrt_sb")
                wrt_view = wrt.rearrange("(c p) s -> p c s", p=P)
                for c in range(Ct):
                    nc.sync.dma_start(
                        wrt_sb[:, c : c + 1, :], wrt_view[:, c : c + 1, :]
                    )
                p_drams, pt_drams, eit_drams, rzcs, rzds = {}, {}, {}, {}, {}
                for b in range(BL):
                    p_dram = dpool.tile([T, S], BF16, tag="p_dram", name=f"pd{b}")
                    # tile-major PT staging: pt2[t][p][s*128+i] = PT[s*128+p, t*128+i]
                    pt_dram = dpool.tile([Tt, P, S], BF16, tag="pt_dram", name=f"ptd{b}")
                    eit_dram = dpool.tile([C, S], BF16, tag="eit_dram", name=f"eitd{b}")
                    rzc = p2.tile([P, Tt], FP32, tag="rzc", name=f"rzc{b}")
                    zdall = p2.tile([P, Tt, St], FP32, tag="zdall", name=f"zdall{b}")
                    rzd = p2.tile([P, St], FP32, tag="rzd", name=f"rzd{b}")
                    p_drams[b], pt_drams[b], eit_drams[b] = p_dram, pt_dram, eit_dram
                    rzcs[b], rzds[b] = rzc, rzd

                    # ---- Phase R: router logits g = x @ wr^T, P = exp(g) ----
                    for t in range(Tt):
                        xbt_t = p3.tile([P, Ct, P], BF16, tag="xbt_t", bufs=2)
                        nc.sync.dma_start(
                            xbt_t[:],
                            xbtp[b, t].rearrange("p (c i) -> p c i", c=Ct),
                        )
                        gps = [pp.tile([P, 512], FP32, tag="ps", name=f"gps{n}") for n in range(4)]
                        for c in range(Ct):
                            for n in range(4):
                                nc.tensor.matmul(
                                    gps[n][:],
                                    xbt_t[:, c, :],
                                    wrt_sb[:, c, n * 512 : (n + 1) * 512],
                                    start=(c == 0),
                                    stop=(c == Ct - 1),
                                )
                        pch = p3.tile([P, S], BF16, tag="pch", bufs=2)
                        zc4 = sp.tile([P, 4], FP32, tag="zc4")
                        for n in range(4):
                            nc.scalar.activation(
                                pch[:, n * 512 : (n + 1) * 512],
                                gps[n][:],
                                mybir.ActivationFunctionType.Exp,
                                accum_out=zc4[:, n : n + 1],
                            )
                        zc1 = sp.tile([P, 1], FP32, tag="zc1")
                        nc.vector.tensor_reduce(zc1[:], zc4[:], mybir.AxisListType.X, AX.add)
                        nc.vector.reciprocal(rzc[:, t : t + 1], zc1[:])
                        nc.gpsimd.dma_start(
                            p_dram[t * P : (t + 1) * P, :], pch[:]
                        )

                    # ---- Phase T: transpose P -> PT (DMA xbar), Zd partials ----
                    for t in range(Tt):
                        ptt = p3.tile([P, St, P], BF16, tag="ptt", bufs=2)
                        nc.scalar.dma_start_transpose(
                            ptt[:], p_dram[t * P : (t + 1) * P, :]
                        )
                        nc.vector.tensor_reduce(
                            zdall[:, t, :], ptt[:], mybir.AxisListType.X, AX.add
                        )
                        nc.gpsimd.dma_start(
                            pt_dram[t].rearrange("p (s i) -> p s i", s=St), ptt[:]
                        )
                    zd = sp.tile([P, St], FP32, tag="zd")
                    nc.vector.tensor_reduce(
                        zd[:], zdall.rearrange("p t s -> p s t"), mybir.AxisListType.X, AX.add
                    )
                    nc.vector.reciprocal(rzd[:], zd[:])

                    # ---- Phase D: dispatch eiT = x^T @ P (unnormalized) ----
                    for nh in range(2):
                        for mg in range(2):
                            dps = [pp.tile([P, 512], FP32, tag="ps", name=f"dps{i}") for i in range(8)]
                            for k in range(Tt):
                                xk = p3.tile([P, 512], BF16, tag="xk")
                                nc.sync.dma_start(
                                    xk[:],
                                    xb[b].rearrange("(k p) c -> p k c", p=P)[
                                        :, k, mg * 512 : (mg + 1) * 512
                                    ],
                                )
                                pk = p3.tile([P, 1024], BF16, tag="pk", bufs=2)
                                nc.sync.dma_start(
                                    pk[:],
                                    p_dram[
                                        k * P : (k + 1) * P,
                                        nh * 1024 : (nh + 1) * 1024,
                                    ],
                                )
                                for m4 in range(4):
                                    for n2 in range(2):
                                        nc.tensor.matmul(
                                            dps[m4 * 2 + n2][:],
                                            xk[:, m4 * P : (m4 + 1) * P],
                                            pk[:, n2 * 512 : (n2 + 1) * 512],
                                            start=(k == 0),
                                            stop=(k == Tt - 1),
                                        )
                            for i in range(8):
                                m = mg * 4 + i // 2
                                n = nh * 2 + i % 2
                                et = p3.tile([P, 512], BF16, tag="et")
                                nc.scalar.copy(et[:], dps[i][:])
                                nc.gpsimd.dma_start(
                                    eit_dram[
                                        m * P : (m + 1) * P, n * 512 : (n + 1) * 512
                                    ],
                                    et[:],
                                )

                # ---- Phase M: per-expert GLU MLP, 1-expert-deep skew ----
                eos = {
                    b: cpool.tile([P, St, C], BF16, tag="big", bufs=2, name=f"eo{b}")
                    for b in range(BL)
                }
                hts = {}  # (e, b) -> transposed h, alive ffn(e)..proj(e)

                def ffn(e):
                    eites = {}
                    for b in range(BL):
                        eit_e = p2.tile([P, Ct, P], BF16, tag="eit_e", bufs=4, name=f"eite{b}")
                        nc.sync.dma_start(
                            eit_e[:],
                            eit_drams[b].rearrange("(c p) s -> p c s", p=P)[
                                :, :, e * P : (e + 1) * P
                            ],
                        )
                        eites[b] = eit_e
                    hs = {
                        b: p2.tile([P, H], BF16, tag="h_sb", bufs=2, name=f"h{b}")
                        for b in range(BL)
                    }
                    for hc in range(4):
                        wgc = p2.tile([P, Ct, 512], BF16, tag="wgc", bufs=2)
                        nc.sync.dma_start(
                            wgc[:],
                            wg[e].rearrange("(c p) h -> p c h", p=P)[
                                :, :, hc * 512 : (hc + 1) * 512
                            ],
                        )
                        wfcc = p2.tile([P, Ct, 512], BF16, tag="wfcc", bufs=2)
                        nc.sync.dma_start(
                            wfcc[:],
                            wf[e].rearrange("(c p) h -> p c h", p=P)[
                                :, :, hc * 512 : (hc + 1) * 512
                            ],
                        )
                        for b in range(BL):
                            gg = pp.tile([P, 512], FP32, tag="ps", name=f"gg{b}")
                            hh = pp.tile([P, 512], FP32, tag="ps", name=f"hh{b}")
                            # share each stationary eit tile: gate+fc per c
                            for c in range(Ct):
                                nc.tensor.matmul(
                                    gg[:], eites[b][:, c, :], wgc[:, c, :],
                                    start=(c == 0), stop=(c == Ct - 1),
                                )
                                nc.tensor.matmul(
                                    hh[:], eites[b][:, c, :], wfcc[:, c, :],
                                    start=(c == 0), stop=(c == Ct - 1),
                                )
                            sg = p3.tile([P, 512], BF16, tag="sg", name=f"sg{b}")
                            nc.scalar.activation(
                                sg[:], gg[:],
                                mybir.ActivationFunctionType.Silu,
                                scale=rzds[b][:, e : e + 1],
                            )
                            nc.vector.scalar_tensor_tensor(
                                hs[b][:, hc * 512 : (hc + 1) * 512],
                                hh[:], rzds[b][:, e : e + 1], sg[:],
                                AX.mult, AX.mult,
                            )
                    for b in range(BL):
                        ht = p2.tile([P, Ht, P], BF16, tag="ht", bufs=4, name=f"ht{b}")
                        nc.scalar.dma_start_transpose(ht[:], hs[b][:])
                        hts[(e, b)] = ht

                def proj(e):
                    for cc2 in range(2):
                        # wp weights in two k-half tiles (SBUF pressure)
                        wpcs = []
                        for kh in range(2):
                            wpc = p2.tile([P, Ht // 2, 512], BF16, tag="wpc", bufs=4)
                            nc.sync.dma_start(
                                wpc[:],
                                wp[e].rearrange("(k p) c -> p k c", p=P)[
                                    :, kh * (Ht // 2) : (kh + 1) * (Ht // 2),
                                    cc2 * 512 : (cc2 + 1) * 512
                                ],
                            )
                            wpcs.append(wpc)
                        for b in range(BL):
                            eop = pp.tile([P, 512], FP32, tag="ps", name=f"eop{b}")
                            for k in range(Ht):
                                nc.tensor.matmul(
                                    eop[:], hts[(e, b)][:, k, :],
                                    wpcs[k // (Ht // 2)][:, k % (Ht // 2), :],
                                    start=(k == 0), stop=(k == Ht - 1),
                                )
                            nc.scalar.copy(
                                eos[b][:, e, cc2 * 512 : (cc2 + 1) * 512], eop[:]
                            )
                    for b in range(BL):
                        del hts[(e, b)]

                for e in range(E):
                    ffn(e)
                    if e >= 1:
                        proj(e - 1)
                proj(E - 1)

                # ---- Phase C: combine y = (P @ eo) * rzc ----
                for b in range(BL):
                    for t in range(Tt):
                        ptr = p2.tile([P, St, P], BF16, tag="ptr")
                        nc.sync.dma_start(
                            ptr[:],
                            pt_drams[b][t].rearrange("p (s i) -> p s i", s=St),
                        )
                        yps = [pp.tile([P, 512], FP32, tag="ps", name=f"yps{cc}") for cc in range(2)]
                        # both 512-col halves under one ptr weight load
                        for e in range(St):
                            for cc in range(2):
                                nc.tensor.matmul(
                                    yps[cc][:],
                                    ptr[:, e, :],
                                    eos[b][:, e, cc * 512 : (cc + 1) * 512],
                                    start=(e == 0),
                                    stop=(e == St - 1),
                                )
                        for cc in range(2):
                            ysb = p3.tile([P, 512], FP32, tag="ysb", bufs=2)
                            nc.vector.tensor_scalar_mul(
                                ysb[:], yps[cc][:], rzcs[b][:, t : t + 1]
                            )
                            nc.gpsimd.dma_start(
                                y[b, t * P : (t + 1) * P, cc * 512 : (cc + 1) * 512],
                                ysb[:],
                            )
    _split_multi_waits(nc)
    return nc


def kernel(x, w_router_gate, w_fc, w_gate, w_proj):
    bf16 = ml_dtypes.bfloat16
    wrt_np = np.ascontiguousarray(
        w_router_gate.reshape(S, C).T
    ).astype(bf16)
    wg_np = w_gate.astype(bf16)
    wf_np = w_fc.astype(bf16)
    wp_np = w_proj.astype(bf16)

    in_maps = []
    for c in range(NCORES):
        xc = x[c * BL : (c + 1) * BL]  # [BL, T, C] fp32
        xb_np = xc.astype(bf16)
        # xbtp[b, t, p, c*128+i] = x[b, t*128+i, c*128+p]
        xbtp_np = np.ascontiguousarray(
            xb_np.reshape(BL, T // P, P, C // P, P).transpose(0, 1, 4, 3, 2)
        ).reshape(BL, T // P, P, C)
        in_maps.append(
            {
                "xb": xb_np,
                "xbtp": xbtp_np,
                "wrt": wrt_np,
                "wg": wg_np,
                "wf": wf_np,
                "wp": wp_np,
            }
        )

    from concourse.bass_utils import run_bass_kernel_spmd

    nc = build_nc()
    res = None
    last_err = None
    for attempt in range(3):
        try:
            res = run_bass_kernel_spmd(nc, in_maps, core_ids=list(range(NCORES)))
            break
        except Exception as e:  # transient NRT_EXEC_UNIT_UNRECOVERABLE on first exec
            last_err = e
    if res is None:
        raise last_err
    y = np.concatenate(
        [res.results[c]["y"] for c in range(NCORES)], axis=0
    ).astype(np.float32)
    return y


if __name__ == "__main__":
    xs = np.random.randn(B, T, C).astype(np.float32)
    print("built", build_nc())
